# revision 5
# baseline (speedup 1.0000x reference)
"""Multi-head attention (B=2, S=2048, d_model=1024, H=16) on 8 TRN2 NeuronCores.

Sharding: core c handles batch b = c//4 and the 4 heads hg*4..hg*4+4 (hg = c%4):
data-parallel over batch, head-parallel within batch. W_Q/W_K/W_V are
column-sharded (output rows per head group), W_O row-sharded; partial outputs
are summed on the host (row-parallel unshard), attention weights concatenated.

Per core, on device (all matmul operands bf16, accumulation fp32):
  QT_h/KT_h = W_slice @ x^T projections ([64, S], head-dim on partitions)
  V         = value @ W_v_slice^T ([S, 4, 64+ones-column] tiles)
  Phase B (attention-weights output): scores[q,k] q-chunks on PE -> ACT exp
      (scale=1/8, accum_out row-sums) -> DVE reciprocal + per-partition
      normalize -> DMA out f32
  Phase A (context): scores^T[k,q] chunks -> ACT exp (bf16) -> ctx^T psum
      accumulation via V|ones (softmax denominators land on partition 64)
      -> selector-matrix replication matmul -> reciprocal -> multiply
  Final: out_partial[s,:] = sum_h ctx_h^T.T @ W_O_slice^T -> DMA out f32
"""

import numpy as np
import ml_dtypes

import concourse.bass as bass
import concourse.mybir as mybir
import concourse.tile as tile
from concourse import bacc
from concourse.bass_utils import run_bass_kernel_spmd

F32 = mybir.dt.float32
BF16 = mybir.dt.bfloat16
AF = mybir.ActivationFunctionType
MUL = mybir.AluOpType.mult

B = 2
S = 2048
DM = 1024
H = 16
DK = 64
NCORES = 8
HPC = 4              # heads per core
DHC = HPC * DK       # 256 head-dims per core
SCALE = 0.125        # 1/sqrt(DK)
SC = S // 128        # 16 chunks of 128 along s
KCH = DM // 128      # 8 contraction chunks for projections

_NC = None
_LAST = None


def _build():
    nc = bacc.Bacc("TRN2", target_bir_lowering=False, debug=False, num_devices=NCORES)

    qT = nc.declare_dram_parameter("qT", [DM, S], BF16, isOutput=False)
    kT = nc.declare_dram_parameter("kT", [DM, S], BF16, isOutput=False)
    vT = nc.declare_dram_parameter("vT", [DM, S], BF16, isOutput=False)
    wq = nc.declare_dram_parameter("wq", [128, KCH, DHC], BF16, isOutput=False)
    wk = nc.declare_dram_parameter("wk", [128, KCH, DHC], BF16, isOutput=False)
    wv = nc.declare_dram_parameter("wv", [128, KCH, DHC], BF16, isOutput=False)
    wo = nc.declare_dram_parameter("wo", [DK, HPC, DM], BF16, isOutput=False)
    attn_d = nc.declare_dram_parameter("attn", [HPC, S, S], F32, isOutput=True)
    outp_d = nc.declare_dram_parameter("outp", [S, DM], F32, isOutput=True)

    with tile.TileContext(nc) as tc:
        with (
            tc.tile_pool(name="persist", bufs=1) as pp,
            tc.tile_pool(name="xin", bufs=9) as xp,
            tc.tile_pool(name="exp", bufs=3) as ep,
            tc.tile_pool(name="attn", bufs=4) as ap_,
            tc.tile_pool(name="sml", bufs=2) as sp,
            tc.tile_pool(name="ps", bufs=1, space=bass.MemorySpace.PSUM) as ps,
        ):
            # ---- constants & weights ----
            wq_t = pp.tile([128, KCH, DHC], BF16, tag="wq")
            wk_t = pp.tile([128, KCH, DHC], BF16, tag="wk")
            wv_t = pp.tile([128, KCH, DHC], BF16, tag="wv")
            wo_t = pp.tile([DK, HPC, DM], BF16, tag="wo")
            nc.sync.dma_start(wq_t[:], wq[:])
            nc.sync.dma_start(wk_t[:], wk[:])
            nc.sync.dma_start(wv_t[:], wv[:])
            nc.sync.dma_start(wo_t[:], wo[:])

            # selector matrix: zeros, row 64 = ones (replicates the denom row)
            e_t = pp.tile([DK + 1, DK], F32, tag="emat")
            nc.vector.memset(e_t[:], 0.0)
            nc.vector.memset(e_t[DK:DK + 1, :], 1.0)

            QTh = [pp.tile([DK, S], BF16, tag=f"QT{h}", name=f"QT{h}") for h in range(HPC)]
            KTh = [pp.tile([DK, S], BF16, tag=f"KT{h}", name=f"KT{h}") for h in range(HPC)]
            Vt = [pp.tile([128, HPC, 66], BF16, tag=f"Vt{i}", name=f"Vt{i}") for i in range(SC)]
            ctx_h = [pp.tile([DK, S], BF16, tag=f"ctx{h}", name=f"ctx{h}") for h in range(HPC)]

            def load_x(dram):
                ts_ = []
                for kc in range(KCH):
                    t = xp.tile([128, S], BF16, tag="xin")
                    nc.sync.dma_start(t[:], dram[kc * 128:(kc + 1) * 128, :])
                    ts_.append(t)
                return ts_

            # ---- Q/K projections: QT_h[64, S] = (W_slice_h @ x^T) ----
            for x_dram, w_t, dst in ((qT, wq_t, QTh), (kT, wk_t, KTh)):
                xt = load_x(x_dram)
                for h in range(HPC):
                    pb = ps.tile([128, S], F32, tag="big")
                    for kc in range(KCH):
                        for nn in range(4):
                            nc.tensor.matmul(
                                pb[0:DK, nn * 512:(nn + 1) * 512],
                                w_t[:, kc, h * DK:(h + 1) * DK],
                                xt[kc][:, nn * 512:(nn + 1) * 512],
                                start=(kc == 0), stop=(kc == KCH - 1),
                            )
                    nc.vector.tensor_copy(dst[h][:], pb[0:DK, :])

            # ---- V projection: V[s, dh] chunks with ones column ----
            vt = load_x(vT)
            for i in range(SC):
                psm = ps.tile([128, 1024], F32, tag="small")
                for kc in range(KCH):
                    nc.tensor.matmul(
                        psm[:, :DHC],
                        vt[kc][:, i * 128:(i + 1) * 128],
                        wv_t[:, kc, :],
                        start=(kc == 0), stop=(kc == KCH - 1),
                    )
                for h in range(HPC):
                    nc.vector.tensor_copy(Vt[i][:, h, :DK], psm[:, h * DK:(h + 1) * DK])
                nc.vector.memset(Vt[i][:, :, DK:DK + 1], 1.0)

            # ---- attention: per head, B-units (attn out) and A-units (ctx)
            # interleaved so PE/ACT/DVE/DMA pipelines stay full ----
            for h in range(HPC):

                def unit_B(qc):
                    pb = ps.tile([128, S], F32, tag="big")
                    for nn in range(4):
                        nc.tensor.matmul(
                            pb[:, nn * 512:(nn + 1) * 512],
                            QTh[h][:, qc * 128:(qc + 1) * 128],
                            KTh[h][:, nn * 512:(nn + 1) * 512],
                            start=True, stop=True,
                        )
                    at = ap_.tile([128, S], F32, tag="at")
                    rs = sp.tile([128, 1], F32, tag="rs")
                    nc.scalar.activation(at[:], pb[:], AF.Exp, scale=SCALE,
                                         accum_out=rs[:])
                    rc = sp.tile([128, 1], F32, tag="rc")
                    nc.vector.reciprocal(rc[:], rs[:])
                    nc.vector.tensor_scalar_mul(at[:], at[:], rc[:])
                    nc.sync.dma_start(attn_d[h, qc * 128:(qc + 1) * 128, :], at[:])

                def unit_A(qh, kc, ctx_ps):
                    psm = ps.tile([128, 1024], F32, tag="small")
                    for nn in range(2):
                        nc.tensor.matmul(
                            psm[:, nn * 512:(nn + 1) * 512],
                            KTh[h][:, kc * 128:(kc + 1) * 128],
                            QTh[h][:, qh * 1024 + nn * 512: qh * 1024 + (nn + 1) * 512],
                            start=True, stop=True,
                        )
                    et = ep.tile([128, 1024], BF16, tag="expT")
                    nc.scalar.activation(et[:], psm[:], AF.Exp, scale=SCALE)
                    for nn in range(2):
                        nc.tensor.matmul(
                            ctx_ps[0:DK + 1, nn * 512:(nn + 1) * 512],
                            Vt[kc][:, h, :DK + 1],
                            et[:, nn * 512:(nn + 1) * 512],
                            start=(kc == 0), stop=(kc == SC - 1),
                        )

                def finish_A(qh, ctx_ps):
                    rr = sp.tile([DK + 1, 1024], F32, tag="rr")
                    nc.vector.memset(rr[0:DK, :], 0.0)
                    nc.vector.tensor_copy(rr[DK:DK + 1, :], ctx_ps[DK:DK + 1, :])
                    rep_ps = ps.tile([128, 1024], F32, tag="small")
                    for nn in range(2):
                        nc.tensor.matmul(
                            rep_ps[0:DK, nn * 512:(nn + 1) * 512],
                            e_t[:], rr[:, nn * 512:(nn + 1) * 512],
                            start=True, stop=True,
                        )
                    rep = sp.tile([DK, 1024], F32, tag="rep")
                    nc.vector.reciprocal_approx_fast(rep[:], rep_ps[0:DK, :])
                    nc.vector.tensor_tensor(
                        ctx_h[h][:, qh * 1024:(qh + 1) * 1024],
                        ctx_ps[0:DK, :], rep[:], MUL,
                    )

                ctx_ps = None
                for i in range(SC):
                    unit_B(i)
                    for j in (2 * i, 2 * i + 1):
                        qh, kc = j // SC, j % SC
                        if kc == 0:
                            ctx_ps = ps.tile([128, 1024], F32, tag="ctx")
                        unit_A(qh, kc, ctx_ps)
                        if kc == SC - 1:
                            finish_A(qh, ctx_ps)

            # ---- final projection: outp = sum_h ctx_h^T.T @ W_O_slice_h^T ----
            for sc in range(SC):
                psm = ps.tile([128, 1024], F32, tag="small")
                for h in range(HPC):
                    for nn in range(2):
                        nc.tensor.matmul(
                            psm[:, nn * 512:(nn + 1) * 512],
                            ctx_h[h][:, sc * 128:(sc + 1) * 128],
                            wo_t[:, h, nn * 512:(nn + 1) * 512],
                            start=(h == 0), stop=(h == HPC - 1),
                        )
                ot = sp.tile([128, 1024], F32, tag="ot")
                nc.vector.tensor_copy(ot[:], psm[:])
                nc.sync.dma_start(outp_d[sc * 128:(sc + 1) * 128, :], ot[:])

    nc.compile()
    return nc


def _get_nc():
    global _NC
    if _NC is None:
        _NC = _build()
    return _NC


def _pack_w(w_slice_T, kch):
    # [kch*P, n] -> [P, kch, n] so each per-partition row is contiguous in HBM
    p = w_slice_T.shape[0] // kch
    return np.ascontiguousarray(
        w_slice_T.reshape(kch, p, -1).transpose(1, 0, 2)
    ).astype(ml_dtypes.bfloat16)


def _kernel_np(query, key, value, mask, W_Q, W_K, W_V, W_O):
    # numpy fallback (only if mask isn't all ones — never per the spec)
    Bq = query.shape[0]
    Q = (query @ W_Q.T).reshape(Bq, -1, H, DK).transpose(0, 2, 1, 3)
    K = (key @ W_K.T).reshape(Bq, -1, H, DK).transpose(0, 2, 1, 3)
    V = (value @ W_V.T).reshape(Bq, -1, H, DK).transpose(0, 2, 1, 3)
    s = np.einsum("bhqd,bhkd->bhqk", Q, K) / np.float32(np.sqrt(DK))
    s = np.where(mask == 0, -np.inf, s)
    s = s - s.max(axis=-1, keepdims=True)
    e = np.exp(s)
    aw = e / e.sum(axis=-1, keepdims=True)
    ctx = np.einsum("bhqk,bhkd->bhqd", aw, V)
    ctx = ctx.transpose(0, 2, 1, 3).reshape(Bq, -1, DM)
    return (ctx @ W_O.T).astype(np.float32), aw.astype(np.float32)


def kernel(query, key, value, mask, W_Q, W_K, W_V, W_O):
    query = np.asarray(query, dtype=np.float32)
    key = np.asarray(key, dtype=np.float32)
    value = np.asarray(value, dtype=np.float32)
    mask = np.asarray(mask)
    W_Q = np.asarray(W_Q, dtype=np.float32)
    W_K = np.asarray(W_K, dtype=np.float32)
    W_V = np.asarray(W_V, dtype=np.float32)
    W_O = np.asarray(W_O, dtype=np.float32)

    if not np.all(mask != 0):
        return _kernel_np(query, key, value, mask, W_Q, W_K, W_V, W_O)

    nc = _get_nc()
    bf = ml_dtypes.bfloat16
    xT = {}
    for b in range(B):
        xT[("q", b)] = np.ascontiguousarray(query[b].T).astype(bf)
        xT[("k", b)] = np.ascontiguousarray(key[b].T).astype(bf)
        xT[("v", b)] = np.ascontiguousarray(value[b].T).astype(bf)

    in_maps = []
    for c in range(NCORES):
        b, hg = c // HPC, c % HPC
        rows = slice(hg * DHC, (hg + 1) * DHC)
        in_maps.append({
            "qT": xT[("q", b)],
            "kT": xT[("k", b)],
            "vT": xT[("v", b)],
            "wq": _pack_w(np.ascontiguousarray(W_Q[rows, :].T), KCH),
            "wk": _pack_w(np.ascontiguousarray(W_K[rows, :].T), KCH),
            "wv": _pack_w(np.ascontiguousarray(W_V[rows, :].T), KCH),
            "wo": _pack_w(np.ascontiguousarray(W_O[:, rows].T), HPC),
        })

    res = run_bass_kernel_spmd(nc, in_maps, list(range(NCORES)))
    global _LAST
    _LAST = res

    attn = np.empty((B, H, S, S), dtype=np.float32)
    output = np.zeros((B, S, DM), dtype=np.float32)
    for c in range(NCORES):
        b, hg = c // HPC, c % HPC
        attn[b, hg * HPC:(hg + 1) * HPC] = res.results[c]["attn"]
        output[b] += res.results[c]["outp"]
    return output, attn


# revision 13
# speedup vs baseline: 1.3336x; 1.3336x over previous
"""Multi-head attention (B=2, S=2048, d_model=1024, H=16) on 8 TRN2 NeuronCores.

Sharding: core c handles batch b = c//4 and the 4 heads hg*4..hg*4+4 (hg = c%4):
data-parallel over batch, head-parallel within batch. W_Q/W_K/W_V are
column-sharded (output rows per head group), W_O row-sharded; partial outputs
are summed on the host (row-parallel unshard), attention weights concatenated.

Per core, on device (all matmul operands bf16, accumulation fp32):
  QT_h/KT_h = W_slice @ x^T projections ([64, S], head-dim on partitions)
  V         = value @ W_v_slice^T ([S, 4, 64+ones-column] tiles)
  Phase B (attention-weights output): scores[q,k] q-chunks on PE -> ACT exp
      (scale=1/8, accum_out row-sums) -> DVE reciprocal + per-partition
      normalize -> DMA out f32
  Phase A (context): scores^T[k,q] chunks -> ACT exp (bf16) -> ctx^T psum
      accumulation via V|ones (softmax denominators land on partition 64)
      -> selector-matrix replication matmul -> reciprocal -> multiply
  Final: out_partial[s,:] = sum_h ctx_h^T.T @ W_O_slice^T -> DMA out f32
"""

import numpy as np
import ml_dtypes

import concourse.bass as bass
import concourse.mybir as mybir
import concourse.tile as tile
from concourse import bacc
from concourse.bass_utils import run_bass_kernel_spmd

F32 = mybir.dt.float32
BF16 = mybir.dt.bfloat16
AF = mybir.ActivationFunctionType
MUL = mybir.AluOpType.mult

B = 2
S = 2048
DM = 1024
H = 16
DK = 64
NCORES = 8
HPC = 4              # heads per core
DHC = HPC * DK       # 256 head-dims per core
SCALE = 0.125        # 1/sqrt(DK)
SC = S // 128        # 16 chunks of 128 along s
KCH = DM // 128      # 8 contraction chunks for projections

_NC = None
_LAST = None


def _build():
    nc = bacc.Bacc("TRN2", target_bir_lowering=False, debug=False, num_devices=NCORES)

    qT = nc.declare_dram_parameter("qT", [DM, S], BF16, isOutput=False)
    kT = nc.declare_dram_parameter("kT", [DM, S], BF16, isOutput=False)
    vT = nc.declare_dram_parameter("vT", [DM, S], BF16, isOutput=False)
    wq = nc.declare_dram_parameter("wq", [128, KCH, DHC], BF16, isOutput=False)
    wk = nc.declare_dram_parameter("wk", [128, KCH, DHC], BF16, isOutput=False)
    wv = nc.declare_dram_parameter("wv", [128, KCH, DHC], BF16, isOutput=False)
    wo = nc.declare_dram_parameter("wo", [DK, HPC, DM], BF16, isOutput=False)
    attn_d = nc.declare_dram_parameter("attn", [HPC, S, S], F32, isOutput=True)
    outp_d = nc.declare_dram_parameter("outp", [S, DM], F32, isOutput=True)

    with tile.TileContext(nc) as tc:
        with (
            tc.tile_pool(name="persist", bufs=1) as pp,
            tc.tile_pool(name="xin", bufs=9) as xp,
            tc.tile_pool(name="exp", bufs=4) as ep,
            tc.tile_pool(name="attn", bufs=4) as ap_,
            tc.tile_pool(name="sml", bufs=3) as sp,
            tc.tile_pool(name="ps", bufs=1, space=bass.MemorySpace.PSUM) as ps,
        ):
            # ---- constants & weights ----
            wq_t = pp.tile([128, KCH, DHC], BF16, tag="wq")
            wk_t = pp.tile([128, KCH, DHC], BF16, tag="wk")
            wv_t = pp.tile([128, KCH, DHC], BF16, tag="wv")
            wo_t = pp.tile([DK, HPC, DM], BF16, tag="wo")
            nc.sync.dma_start(wq_t[:], wq[:])
            nc.sync.dma_start(wk_t[:], wk[:])
            nc.sync.dma_start(wv_t[:], wv[:])
            nc.sync.dma_start(wo_t[:], wo[:])

            # selector matrix: zeros, row 64 = ones (replicates the denom row)
            e_t = pp.tile([DK + 1, DK], F32, tag="emat")
            nc.vector.memset(e_t[:], 0.0)
            nc.vector.memset(e_t[DK:DK + 1, :], 1.0)

            QTh = [pp.tile([DK, S], BF16, tag=f"QT{h}", name=f"QT{h}") for h in range(HPC)]
            KTh = [pp.tile([DK, S], BF16, tag=f"KT{h}", name=f"KT{h}") for h in range(HPC)]
            Vt = [pp.tile([128, HPC, 66], BF16, tag=f"Vt{i}", name=f"Vt{i}") for i in range(SC)]
            ctx_h = [pp.tile([DK, S], BF16, tag=f"ctx{h}", name=f"ctx{h}") for h in range(HPC)]

            def load_x(dram):
                ts_ = []
                for kc in range(KCH):
                    t = xp.tile([128, S], BF16, tag="xin")
                    nc.sync.dma_start(t[:], dram[kc * 128:(kc + 1) * 128, :])
                    ts_.append(t)
                return ts_

            # ---- Q/K projections: QT_h[64, S] = (W_slice_h @ x^T) ----
            for x_dram, w_t, dst in ((qT, wq_t, QTh), (kT, wk_t, KTh)):
                xt = load_x(x_dram)
                for h in range(HPC):
                    for half in range(2):
                        pb = ps.tile([128, 1024], F32, tag="ps1", bufs=3)
                        for kc in range(KCH):
                            for nn in range(2):
                                nc.tensor.matmul(
                                    pb[0:DK, nn * 512:(nn + 1) * 512],
                                    w_t[:, kc, h * DK:(h + 1) * DK],
                                    xt[kc][:, half * 1024 + nn * 512:
                                           half * 1024 + (nn + 1) * 512],
                                    start=(kc == 0), stop=(kc == KCH - 1),
                                )
                        nc.vector.tensor_copy(
                            dst[h][:, half * 1024:(half + 1) * 1024], pb[0:DK, :])

            # ---- V projection: V[s, dh] chunks with ones column ----
            vt = load_x(vT)
            for i in range(SC):
                psm = ps.tile([128, 1024], F32, tag="ps1", bufs=3)
                for kc in range(KCH):
                    nc.tensor.matmul(
                        psm[:, :DHC],
                        vt[kc][:, i * 128:(i + 1) * 128],
                        wv_t[:, kc, :],
                        start=(kc == 0), stop=(kc == KCH - 1),
                    )
                for h in range(HPC):
                    nc.vector.tensor_copy(Vt[i][:, h, :DK], psm[:, h * DK:(h + 1) * DK])
                nc.vector.memset(Vt[i][:, :, DK:DK + 1], 1.0)

            # ---- attention: per head, B-units (attn out) and A-units (ctx)
            # interleaved so PE/ACT/DVE/DMA pipelines stay full ----
            for h in range(HPC):

                def unit_B(qc):
                    at = ap_.tile([128, S], F32, tag="at")
                    rsh = []
                    for half in range(2):
                        pb = ps.tile([128, 1024], F32, tag="ps1", bufs=3)
                        for nn in range(2):
                            nc.tensor.matmul(
                                pb[:, nn * 512:(nn + 1) * 512],
                                QTh[h][:, qc * 128:(qc + 1) * 128],
                                KTh[h][:, half * 1024 + nn * 512:
                                       half * 1024 + (nn + 1) * 512],
                                start=True, stop=True,
                            )
                        rs = sp.tile([128, 1], F32, tag=f"rs{half}",
                                     name=f"rs{half}")
                        nc.scalar.activation(
                            at[:, half * 1024:(half + 1) * 1024], pb[:],
                            AF.Exp, scale=SCALE, accum_out=rs[:])
                        rsh.append(rs)
                    rc = sp.tile([128, 1], F32, tag="rc")
                    nc.vector.tensor_tensor(rc[:], rsh[0][:], rsh[1][:],
                                            mybir.AluOpType.add)
                    nc.vector.reciprocal(rc[:], rc[:])
                    nc.vector.tensor_scalar_mul(at[:], at[:], rc[:])
                    nc.sync.dma_start(attn_d[h, qc * 128:(qc + 1) * 128, :], at[:])

                def unit_A(qh, kc, ctx_ps):
                    psm = ps.tile([128, 1024], F32, tag="ps1", bufs=3)
                    for nn in range(2):
                        nc.tensor.matmul(
                            psm[:, nn * 512:(nn + 1) * 512],
                            KTh[h][:, kc * 128:(kc + 1) * 128],
                            QTh[h][:, qh * 1024 + nn * 512: qh * 1024 + (nn + 1) * 512],
                            start=True, stop=True,
                        )
                    et = ep.tile([128, 1024], BF16, tag="expT")
                    nc.scalar.activation(et[:], psm[:], AF.Exp, scale=SCALE)
                    for nn in range(2):
                        nc.tensor.matmul(
                            ctx_ps[0:DK + 1, nn * 512:(nn + 1) * 512],
                            Vt[kc][:, h, :DK + 1],
                            et[:, nn * 512:(nn + 1) * 512],
                            start=(kc == 0), stop=(kc == SC - 1),
                        )

                def finish_A(qh, ctx_ps):
                    rr = sp.tile([DK + 1, 1024], F32, tag="rr")
                    nc.vector.memset(rr[0:DK, :], 0.0)
                    nc.vector.tensor_copy(rr[DK:DK + 1, :], ctx_ps[DK:DK + 1, :])
                    rep_ps = ps.tile([128, 1024], F32, tag="ps1", bufs=3)
                    for nn in range(2):
                        nc.tensor.matmul(
                            rep_ps[0:DK, nn * 512:(nn + 1) * 512],
                            e_t[:], rr[:, nn * 512:(nn + 1) * 512],
                            start=True, stop=True,
                        )
                    rep = sp.tile([DK, 1024], F32, tag="rep")
                    nc.vector.reciprocal_approx_fast(rep[:], rep_ps[0:DK, :])
                    nc.vector.tensor_tensor(
                        ctx_h[h][:, qh * 1024:(qh + 1) * 1024],
                        ctx_ps[0:DK, :], rep[:], MUL,
                    )

                ctx_ps = None
                for i in range(SC):
                    unit_B(i)
                    for j in (2 * i, 2 * i + 1):
                        qh, kc = j // SC, j % SC
                        if kc == 0:
                            ctx_ps = ps.tile([128, 1024], F32, tag="ctx")
                        unit_A(qh, kc, ctx_ps)
                        if kc == SC - 1:
                            finish_A(qh, ctx_ps)

            # ---- final projection: outp = sum_h ctx_h^T.T @ W_O_slice_h^T ----
            for sc in range(SC):
                psm = ps.tile([128, 1024], F32, tag="ps1", bufs=3)
                for h in range(HPC):
                    for nn in range(2):
                        nc.tensor.matmul(
                            psm[:, nn * 512:(nn + 1) * 512],
                            ctx_h[h][:, sc * 128:(sc + 1) * 128],
                            wo_t[:, h, nn * 512:(nn + 1) * 512],
                            start=(h == 0), stop=(h == HPC - 1),
                        )
                ot = sp.tile([128, 1024], F32, tag="ot")
                nc.vector.tensor_copy(ot[:], psm[:])
                nc.sync.dma_start(outp_d[sc * 128:(sc + 1) * 128, :], ot[:])

    nc.compile()
    return nc


def _get_nc():
    global _NC
    if _NC is None:
        _NC = _build()
    return _NC


def _pack_w(w_slice_T, kch):
    # [kch*P, n] -> [P, kch, n] so each per-partition row is contiguous in HBM
    p = w_slice_T.shape[0] // kch
    return np.ascontiguousarray(
        w_slice_T.reshape(kch, p, -1).transpose(1, 0, 2)
    ).astype(ml_dtypes.bfloat16)


def _kernel_np(query, key, value, mask, W_Q, W_K, W_V, W_O):
    # numpy fallback (only if mask isn't all ones — never per the spec)
    Bq = query.shape[0]
    Q = (query @ W_Q.T).reshape(Bq, -1, H, DK).transpose(0, 2, 1, 3)
    K = (key @ W_K.T).reshape(Bq, -1, H, DK).transpose(0, 2, 1, 3)
    V = (value @ W_V.T).reshape(Bq, -1, H, DK).transpose(0, 2, 1, 3)
    s = np.einsum("bhqd,bhkd->bhqk", Q, K) / np.float32(np.sqrt(DK))
    s = np.where(mask == 0, -np.inf, s)
    s = s - s.max(axis=-1, keepdims=True)
    e = np.exp(s)
    aw = e / e.sum(axis=-1, keepdims=True)
    ctx = np.einsum("bhqk,bhkd->bhqd", aw, V)
    ctx = ctx.transpose(0, 2, 1, 3).reshape(Bq, -1, DM)
    return (ctx @ W_O.T).astype(np.float32), aw.astype(np.float32)


def kernel(query, key, value, mask, W_Q, W_K, W_V, W_O):
    query = np.asarray(query, dtype=np.float32)
    key = np.asarray(key, dtype=np.float32)
    value = np.asarray(value, dtype=np.float32)
    mask = np.asarray(mask)
    W_Q = np.asarray(W_Q, dtype=np.float32)
    W_K = np.asarray(W_K, dtype=np.float32)
    W_V = np.asarray(W_V, dtype=np.float32)
    W_O = np.asarray(W_O, dtype=np.float32)

    if not np.all(mask != 0):
        return _kernel_np(query, key, value, mask, W_Q, W_K, W_V, W_O)

    nc = _get_nc()
    bf = ml_dtypes.bfloat16
    xT = {}
    for b in range(B):
        xT[("q", b)] = np.ascontiguousarray(query[b].T).astype(bf)
        xT[("k", b)] = np.ascontiguousarray(key[b].T).astype(bf)
        xT[("v", b)] = np.ascontiguousarray(value[b].T).astype(bf)

    in_maps = []
    for c in range(NCORES):
        b, hg = c // HPC, c % HPC
        rows = slice(hg * DHC, (hg + 1) * DHC)
        in_maps.append({
            "qT": xT[("q", b)],
            "kT": xT[("k", b)],
            "vT": xT[("v", b)],
            "wq": _pack_w(np.ascontiguousarray(W_Q[rows, :].T), KCH),
            "wk": _pack_w(np.ascontiguousarray(W_K[rows, :].T), KCH),
            "wv": _pack_w(np.ascontiguousarray(W_V[rows, :].T), KCH),
            "wo": _pack_w(np.ascontiguousarray(W_O[:, rows].T), HPC),
        })

    res = run_bass_kernel_spmd(nc, in_maps, list(range(NCORES)))
    global _LAST
    _LAST = res

    attn = np.empty((B, H, S, S), dtype=np.float32)
    output = np.zeros((B, S, DM), dtype=np.float32)
    for c in range(NCORES):
        b, hg = c // HPC, c % HPC
        attn[b, hg * HPC:(hg + 1) * HPC] = res.results[c]["attn"]
        output[b] += res.results[c]["outp"]
    return output, attn
